# revision 21
# baseline (speedup 1.0000x reference)
"""CRF layer (Viterbi decode + log-likelihood loss) as a Bass/Trainium2 kernel.

Strategy: pure data parallel over 8 NeuronCores. Host sorts sequences by
length (descending) and deals them round-robin so each core gets the same
length profile: slot0 = its 128 longest (loop bound L0 = global max len),
slot1 = its 128 shortest (bound L1 = 1025th-longest len). Steps beyond a
slot's bound are pure padding for every sequence in it and are skipped.

Per core, per slot (exact-fp32 Viterbi, bit-matching the jax reference):
  fwd:  scores[b,(j,i)] = delta[b,i] + T[i,j]   (DVE tensor_tensor, bcast AP)
        m[b,j] = max_i scores                   (DVE 3D reduce_max)
        delta'[b,j] = logit_t[b,j] + m[b,j], frozen at t>=len via select.
        delta overwrites the logits tile in place (history kept for bwd).
  loss: scaled sum-space forward algorithm: v <- (expT_blockdiag^T v) * exp(l_t)
        on PE (v packed [(group,i) x 32b] via DVE 32x32-block transpose),
        log-partition captured at t=len-1 with a masked select.
  bwd:  tag_t = argmax_i(delta_t[i] + T[i, tag_{t+1}]) with first-index
        tie-break, via GPSIMD indirect gather of T's column + DVE argmax.

Outputs gathered on host: unpermute tags, ll = unary+binary-logZ, loss=mean.
"""

import sys
from contextlib import ExitStack

import numpy as np

sys.path.insert(0, "/opt/trn_rl_repo")

import concourse.bacc as bacc  # noqa: E402
import concourse.bass as bass  # noqa: E402
import concourse.tile as tile  # noqa: E402
from concourse import mybir  # noqa: E402

B, T, C = 2048, 512, 32
NCORES = 8
P = 128  # partitions / sequences per slot
ALU = mybir.AluOpType
DT = mybir.dt
NEG = -3.0e38


def _bcast(ap2d, n):
    """[P, F] AP -> [P, n, F] with the n-dim a stride-0 broadcast."""
    return bass.AP(
        tensor=ap2d.tensor,
        offset=ap2d.offset,
        ap=[ap2d.ap[0], [0, n]] + list(ap2d.ap[1:]),
    )


def _bcol(ap_col, n):
    """[P, 1] AP -> [P, n] stride-0 broadcast of the single column."""
    return bass.AP(
        tensor=ap_col.tensor,
        offset=ap_col.offset,
        ap=[ap_col.ap[0], [0, n]],
    )


def build_program(L0, L1, min0, min1, renorm_every=4, stage=4):
    """Emit the full per-core program. Same NEFF runs SPMD on all 8 cores."""
    nc = bacc.Bacc(
        "TRN2", target_bir_lowering=False, debug=False, num_devices=NCORES
    )
    f32, i32, u16, u8 = DT.float32, DT.int32, DT.uint16, DT.uint8

    Ls = [L0, L1]
    mins = [min0, min1]

    # ---- DRAM I/O ----
    ins = {}
    for s, L in enumerate(Ls):
        ins[f"logits{s}"] = nc.dram_tensor(f"logits{s}", [P, L * C], f32,
                                           kind="ExternalInput").ap()
        ins[f"mask{s}"] = nc.dram_tensor(f"mask{s}", [P, L], f32,
                                         kind="ExternalInput").ap()
        ins[f"maskp{s}"] = nc.dram_tensor(f"maskp{s}", [L, P, C], u8,
                                          kind="ExternalInput").ap()
        ins[f"uoh{s}"] = nc.dram_tensor(f"uoh{s}", [P, L * C], u8,
                                        kind="ExternalInput").ap()
        ins[f"bcnt{s}"] = nc.dram_tensor(f"bcnt{s}", [P, C * C], f32,
                                        kind="ExternalInput").ap()
        ins[f"minv{s}"] = nc.dram_tensor(f"minv{s}", [P, L], u8,
                                         kind="ExternalInput").ap()
    ins["transTb"] = nc.dram_tensor("transTb", [P, C * C], f32,
                                    kind="ExternalInput").ap()
    ins["eTbd"] = nc.dram_tensor("eTbd", [P, P], f32,
                                 kind="ExternalInput").ap()
    ins["onesbd"] = nc.dram_tensor("onesbd", [P, P], f32,
                                   kind="ExternalInput").ap()
    ins["iotaj"] = nc.dram_tensor("iotaj", [P, C], f32,
                                  kind="ExternalInput").ap()
    ins["transT32"] = nc.dram_tensor("transT32", [C, C], f32,
                                     kind="ExternalInput").ap()
    ins["ident"] = nc.dram_tensor("ident", [P, P], f32,
                                  kind="ExternalInput").ap()

    outs = {}
    for s, L in enumerate(Ls):
        outs[f"tags{s}"] = nc.dram_tensor(f"tags{s}", [P, L], i32,
                                          kind="ExternalOutput").ap()
        outs[f"ub{s}"] = nc.dram_tensor(f"ub{s}", [P, 1], f32,
                                        kind="ExternalOutput").ap()
        outs[f"lnz{s}"] = nc.dram_tensor(f"lnz{s}", [P, C], f32,
                                         kind="ExternalOutput").ap()

    with tile.TileContext(nc) as tc, ExitStack() as ctx:
        v = nc.vector
        g = nc.gpsimd
        a = nc.scalar
        pe = nc.tensor

        singles = ctx.enter_context(tc.tile_pool(name="singles", bufs=1))
        work = ctx.enter_context(tc.tile_pool(name="work", bufs=3))
        psum = ctx.enter_context(tc.tile_pool(name="psum", bufs=2,
                                              space="PSUM"))
        mpool = ctx.enter_context(tc.tile_pool(name="mp", bufs=2))
        lsep = ctx.enter_context(tc.tile_pool(name="lsep", bufs=3))
        h01p = ctx.enter_context(tc.tile_pool(name="h01p", bufs=3))

        # ---- constants into SBUF ----
        transTb = singles.tile([P, C * C], f32, tag="transTb")
        nc.sync.dma_start(out=transTb, in_=ins["transTb"])
        eTbd = singles.tile([P, P], f32, tag="eTbd")
        nc.sync.dma_start(out=eTbd, in_=ins["eTbd"])
        onesbd = singles.tile([P, P], f32, tag="onesbd")
        nc.sync.dma_start(out=onesbd, in_=ins["onesbd"])
        posidx = singles.tile([P, C], f32, tag="posidx")
        nc.sync.dma_start(out=posidx, in_=ins["iotaj"])
        transT32 = singles.tile([C, C], f32, tag="transT32")
        nc.sync.dma_start(out=transT32, in_=ins["transT32"])
        ident = singles.tile([P, P], f32, tag="ident")
        nc.sync.dma_start(out=ident, in_=ins["ident"])

        for s, (L, minl) in enumerate(zip(Ls, mins)):
            # ---- load slot data ----
            dlog = singles.tile([P, L * C], f32, tag=f"dlog{s}")
            nc.sync.dma_start(out=dlog, in_=ins[f"logits{s}"])
            maskt = singles.tile([P, L], f32, tag=f"mask{s}")
            nc.sync.dma_start(out=maskt, in_=ins[f"mask{s}"])
            minv = singles.tile([P, L], u8, tag=f"minv{s}")
            nc.sync.dma_start(out=minv, in_=ins[f"minv{s}"])

            def dcol(t):
                return dlog[:, t * C:(t + 1) * C]

            # ---- gold score (reads logits before overwrite) ----
            UCH = 2048
            nch = (L * C + UCH - 1) // UCH
            psums = singles.tile([P, nch], f32, tag=f"psums{s}")
            for k in range(nch):
                lo, hi = k * UCH, min((k + 1) * UCH, L * C)
                uchunk = work.tile([P, UCH], u8, tag="uchunk")
                nc.sync.dma_start(out=uchunk[:, :hi - lo],
                                  in_=ins[f"uoh{s}"][:, lo:hi])
                usc = work.tile([P, UCH], f32, tag="usc")
                v.tensor_tensor(usc[:, :hi - lo], dlog[:, lo:hi],
                                uchunk[:, :hi - lo], ALU.mult)
                v.tensor_reduce(psums[:, k:k + 1], usc[:, :hi - lo],
                                mybir.AxisListType.X, ALU.add)
            unary = singles.tile([P, 1], f32, tag=f"unary{s}")
            v.tensor_reduce(unary, psums, mybir.AxisListType.X, ALU.add)
            bcnt = singles.tile([P, C * C], f32, tag=f"bcnt{s}")
            nc.sync.dma_start(out=bcnt, in_=ins[f"bcnt{s}"])
            binary = singles.tile([P, 1], f32, tag=f"binary{s}")
            bsc = work.tile([P, C * C], f32, tag="usc")
            v.tensor_tensor(bsc, bcnt, transTb, ALU.mult)
            v.tensor_reduce(binary, bsc, mybir.AxisListType.X, ALU.add)
            ub = work.tile([P, 1], f32, tag="ub")
            v.tensor_tensor(ub, unary, binary, ALU.add)
            nc.sync.dma_start(out=outs[f"ub{s}"], in_=ub)

            if stage < 2:
                lnzfin0 = singles.tile([P, C], f32, tag=f'lz{s}')
                v.memset(lnzfin0, 0.0)
                nc.sync.dma_start(out=outs[f'lnz{s}'], in_=lnzfin0)
                tgz = singles.tile([P, L], i32, tag=f'tagsi{s}')
                v.memset(tgz, 0)
                nc.sync.dma_start(out=outs[f'tags{s}'], in_=tgz)
                continue
            # ---- LSE state ----
            lnzfin = singles.tile([P, C], f32, tag=f"lz{s}")

            # t=0 init: v0 = exp(logitT_0), renormalized
            ltp = work.tile([P, C], f32, tag="ltp")
            v.transpose(ltp, dcol(0))
            elog = work.tile([P, C], f32, tag="elog")
            a.activation(elog, ltp, mybir.ActivationFunctionType.Exp)
            ps_s = psum.tile([P, C], f32, tag="ps_s")
            pe.matmul(ps_s, onesbd, elog, start=True, stop=True)
            rs = work.tile([P, C], f32, tag="rs")
            v.reciprocal(rs, ps_s)
            vv = lsep.tile([P, C], f32, tag="vv")
            g.tensor_mul(vv, elog, rs)
            lnzcur = lsep.tile([P, C], f32, tag="lc")
            a.activation(lnzcur, ps_s, mybir.ActivationFunctionType.Ln)
            v.tensor_copy(lnzfin, lnzcur)
            rbase = lsep.tile([P, C], f32, tag="rb")
            g.tensor_copy(rbase, lnzcur)

            # maskpack chunk streaming
            CH = 64
            mp_dram = ins[f"maskp{s}"].rearrange("t p c -> p t c")

            scoreb = None
            for t in range(1, L):
                # ======== LSE step t (reads logit col t before overwrite) ==
                ltp = work.tile([P, C], f32, tag="ltp")
                v.transpose(ltp, dcol(t))
                elog = work.tile([P, C], f32, tag="elog")
                a.activation(elog, ltp, mybir.ActivationFunctionType.Exp)
                ps_u = psum.tile([P, C], f32, tag="ps_u")
                pe.matmul(ps_u, eTbd, vv, start=True, stop=True)
                vp = lsep.tile([P, C], f32, tag="vp")
                v.tensor_tensor(vp, ps_u, elog, ALU.mult)  # PSUM src on DVE
                ps_s = psum.tile([P, C], f32, tag="ps_s")
                pe.matmul(ps_s, onesbd, vp, start=True, stop=True)
                lns = work.tile([P, C], f32, tag="lns")
                a.activation(lns, ps_s, mybir.ActivationFunctionType.Ln)
                lnzcur = lsep.tile([P, C], f32, tag="lc")
                g.tensor_add(lnzcur, lns, rbase)
                if t == 1 or t % CH == 0:
                    t0 = (t // CH) * CH
                    mchunk = mpool.tile([P, CH, C], u8, tag="mchunk")
                    n = min(CH, L - t0)
                    nc.sync.dma_start(out=mchunk[:, :n, :],
                                      in_=mp_dram[:, t0:t0 + n, :])
                v.copy_predicated(lnzfin, mchunk[:, t % CH, :], lnzcur)
                if t % renorm_every == 0 or t == L - 1:
                    rs = work.tile([P, C], f32, tag="rs")
                    v.reciprocal(rs, ps_s)
                    vv = lsep.tile([P, C], f32, tag="vv")
                    g.tensor_mul(vv, vp, rs)
                    rbase = lsep.tile([P, C], f32, tag="rb")
                    g.tensor_copy(rbase, lnzcur)
                else:
                    vv = vp  # unnormalized carry; renorm_every bounds growth

                if stage < 3:
                    continue
                # ======== Viterbi step t ========
                scoreb = work.tile([P, C, C], f32, tag="scoreb")
                v.tensor_tensor(scoreb, _bcast(dcol(t - 1), C),
                                transTb[:, :].rearrange("p (j i) -> p j i",
                                                        i=C), ALU.add)
                m32 = work.tile([P, C], f32, tag="m32")
                v.tensor_reduce(m32, scoreb, mybir.AxisListType.X, ALU.max)
                v.tensor_tensor(dcol(t), dcol(t), m32, ALU.add)
                if t >= minl:
                    v.copy_predicated(dcol(t), _bcol(minv[:, t:t + 1], C),
                                      dcol(t - 1))

            nc.sync.dma_start(out=outs[f"lnz{s}"], in_=lnzfin)

            if stage < 4:
                tgz = singles.tile([P, L], i32, tag=f'tagsi{s}')
                v.memset(tgz, 0)
                nc.sync.dma_start(out=outs[f'tags{s}'], in_=tgz)
                continue
            # ---- backward ----
            tagh = singles.tile([P, L], f32, tag=f"tagh{s}")

            def argmax_first(col, ps_in, tagcol):
                """h01 = one-hot of first argmax of (col [+ ps_in]);
                writes tag value into tagcol."""
                if ps_in is None:
                    sc = col
                else:
                    sc = work.tile([P, C], f32, tag="sc")
                    v.tensor_tensor(sc, col, ps_in, ALU.add)
                mm = work.tile([P, 1], f32, tag="m1")
                v.tensor_reduce(mm, sc, mybir.AxisListType.X, ALU.max)
                e01 = work.tile([P, C], f32, tag="e01")
                v.scalar_tensor_tensor(e01, sc, mm, posidx,
                                       ALU.is_equal, ALU.mult)
                qq = work.tile([P, 1], f32, tag="qq")
                v.tensor_reduce(qq, e01, mybir.AxisListType.X, ALU.max)
                h01 = h01p.tile([P, C], f32, tag="h01")
                v.tensor_scalar(h01, e01, qq, None, ALU.is_equal)
                v.tensor_scalar(tagcol, qq, -1.0, float(C), ALU.mult, ALU.add)
                return h01

            def h_from(h01):
                ps_h = psum.tile([C, P], f32, tag="ps_h")
                pe.transpose(ps_h, h01, ident)
                hsb = h01p.tile([C, P], f32, tag="hsb")
                a.copy(hsb, ps_h)
                return hsb

            h01 = argmax_first(dcol(L - 1), None, tagh[:, L - 1:L])
            hsb = h_from(h01)
            for t in range(L - 2, -1, -1):
                ps_t = psum.tile([P, C], f32, tag="ps_t")
                pe.matmul(ps_t, hsb, transT32, start=True, stop=True)
                h01n = argmax_first(dcol(t), ps_t, tagh[:, t:t + 1])
                v.copy_predicated(h01n, _bcol(minv[:, t + 1:t + 2], C), h01)
                v.copy_predicated(tagh[:, t:t + 1], minv[:, t + 1:t + 2],
                                  tagh[:, t + 1:t + 2])
                h01 = h01n
                hsb = h_from(h01)

            # tags masked to 0 beyond len, cast to int32
            tagsm = singles.tile([P, L], f32, tag=f"tagsm{s}")
            v.tensor_tensor(tagsm, tagh, maskt, ALU.mult)
            tagsi = singles.tile([P, L], i32, tag=f"tagsi{s}")
            v.tensor_copy(tagsi, tagsm)
            nc.sync.dma_start(out=outs[f"tags{s}"], in_=tagsi)

    nc.compile()
    return nc


def _host_prep(logits, seq_len, labels, transitions):
    """Sort + shard + build all per-core input tensors."""
    order = np.argsort(-seq_len, kind="stable")
    top, bot = order[:NCORES * P], order[NCORES * P:]
    L0 = int(seq_len[top[0]])
    L1 = int(seq_len[bot[0]]) if len(bot) else 1
    L0 = max(L0, 2)
    L1 = max(L1, 2)
    # slot mins across ALL cores (for select-skip): global min within slot
    min0 = int(seq_len[top[-1]])
    min1 = int(seq_len[bot[-1]])

    tT = np.ascontiguousarray(transitions.T)  # [j, i] -> value T[i,j]
    transTb = np.tile(tT.reshape(1, C * C), (P, 1)).astype(np.float32)
    eT = np.exp(transitions.astype(np.float64)).astype(np.float32)
    eTbd = np.zeros((P, P), np.float32)
    onesbd = np.zeros((P, P), np.float32)
    for gi in range(4):
        eTbd[gi * C:(gi + 1) * C, gi * C:(gi + 1) * C] = eT
        onesbd[gi * C:(gi + 1) * C, gi * C:(gi + 1) * C] = 1.0
    iotaj = np.tile(float(C) - np.arange(C, dtype=np.float32), (P, 1))
    transT32 = np.ascontiguousarray(transitions.T).astype(np.float32)
    ident = np.eye(P, dtype=np.float32)

    in_maps = []
    perms = []
    for c in range(NCORES):
        rows0 = top[c::NCORES]
        rows1 = bot[c::NCORES]
        perms.append((rows0, rows1))
        m = {"transTb": transTb, "eTbd": eTbd, "onesbd": onesbd,
             "iotaj": iotaj, "transT32": transT32, "ident": ident}
        for s, (rows, L) in enumerate([(rows0, L0), (rows1, L1)]):
            lg = np.ascontiguousarray(
                logits[rows, :L, :].reshape(P, L * C)).astype(np.float32)
            ln = seq_len[rows]
            tgrid = np.arange(L)[None, :]
            mask = (tgrid < ln[:, None]).astype(np.float32)
            lab = labels[rows, :L]
            uoh = (np.eye(C, dtype=np.uint8)[lab]
                   * mask[..., None].astype(np.uint8)).reshape(P, L * C)
            pmask = (tgrid[:, :L - 1] < (ln[:, None] - 1)).astype(np.float32)
            bcnt = np.zeros((P, C * C), np.float32)
            np.add.at(bcnt,
                      (np.arange(P)[:, None].repeat(L - 1, 1),
                       lab[:, 1:] * C + lab[:, :-1]), pmask)
            # packed mask for LSE capture: [t, (g,i), b32] = mask[g*32+b32, t]
            mp = mask.T.reshape(L, 4, 1, C).repeat(C, axis=2)
            mp = mp.reshape(L, P, C).astype(np.uint8)
            m[f"logits{s}"] = lg
            m[f"mask{s}"] = np.ascontiguousarray(mask)
            m[f"maskp{s}"] = np.ascontiguousarray(mp)
            m[f"uoh{s}"] = np.ascontiguousarray(uoh)
            m[f"bcnt{s}"] = bcnt
            m[f"minv{s}"] = np.ascontiguousarray(
                (1 - mask).astype(np.uint8))
        in_maps.append(m)
    return in_maps, perms, (L0, L1, min0, min1)


def kernel(logits, sequence_length, labels, transitions, _time_runs=False):
    logits = np.asarray(logits, np.float32)
    seq_len = np.asarray(sequence_length, np.int32)
    labels = np.asarray(labels, np.int32)
    transitions = np.asarray(transitions, np.float32)
    labels = np.maximum(labels, 0)

    from concourse import bass_utils

    in_maps, perms, (L0, L1, min0, min1) = _host_prep(
        logits, seq_len, labels, transitions)
    nc = build_program(L0, L1, min0, min1)
    import os as _os
    import time as _time

    def _run():
        try:
            return bass_utils.run_bass_kernel_spmd(
                nc, in_maps, core_ids=list(range(NCORES)))
        except ModuleNotFoundError:
            # axon NTFF profile hook unavailable; retry untraced
            _os.environ["BASS_NEVER_TRACE"] = "1"
            return bass_utils.run_bass_kernel_spmd(
                nc, in_maps, core_ids=list(range(NCORES)))

    res = _run()
    if res.exec_time_ns is not None:
        print(f"HW exec time: {res.exec_time_ns} ns")
    elif _time_runs:
        t0 = _time.time()
        res = _run()
        dt = time_ns = int((_time.time() - t0) * 1e9)
        print(f"HW exec time: {time_ns} ns (warm wall-clock upper bound)")

    tags = np.zeros((B, T), np.int32)
    ll = np.zeros((B,), np.float64)
    for c in range(NCORES):
        r = res.results[c]
        rows0, rows1 = perms[c]
        for s, (rows, L) in enumerate([(rows0, L0), (rows1, L1)]):
            tags[rows, :L] = r[f"tags{s}"]
            lnz = r[f"lnz{s}"]  # [(g,i), b32]; row i=0 of each group
            logz = lnz.reshape(4, C, C)[:, 0, :].reshape(P)
            ll[rows] = r[f"ub{s}"][:, 0].astype(np.float64) - logz
    loss = np.float32(-(ll.mean()))
    return tags, loss
